# revision 16
# baseline (speedup 1.0000x reference)
"""Trainium2 Bass kernel for a 2-layer LSTM + dense head (batch-sharded over 8 cores).

Reference computation (PyTorch gate order i,f,g,o):
  h1 = LSTM(x;   w_ih1, w_hh1, b_ih1+b_hh1)   # D=128 -> H1=128
  h2 = LSTM(h1;  w_ih2, w_hh2, b_ih2+b_hh2)   # H1=128 -> H2=64
  out = relu(h2[:, -1] @ w_dense.T + b_dense) # [B, 64]

Device-side design (per core, B_c = 256 batch columns):
  - All state "transposed": hidden-dim on SBUF partitions, batch on free dim.
  - fp16 everywhere in SBUF (x, weights, states, gate outputs); fp32 in PSUM.
    fp16 matmuls run at 1 cycle/row; fp16 doubles DVE tensor_tensor rate.
  - Gates i,f,o use Sigmoid directly; the g gate's tanh is computed as
    tanh(z) = 2*sigmoid(2z) - 1 (g weights doubled on host), so ALL four
    gates go through a single Sigmoid ACT op per layer, and every
    elementwise op is a 2-input tensor_tensor (fp16 2x on DVE):
      sg    = sigmoid(psum[i | f | o | 2g])   # one ACT op [128,1024]
      gt    = 2*sg_g - 1                      # tensor_scalar = tanh(g)
      v     = sg_f * c ;  u = sg_i * gt ;  c' = u + v
      thc   = tanh(c') ;  h' = sg_o * thc
    (Sigmoid, Tanh, Relu all live in one HW activation table -> no reloads.)
  - Layer 2 runs one timestep BEHIND layer 1 (software pipeline) so the two
    recurrence chains overlap.  Layer 2's four 64-row gates are packed
    pairwise into two 128-row matmuls: colA = [i | 2g], colB = [f | o].
    Its v2 multiply runs on GPSIMD to keep the DVE/ACT queues tight.
  - Layer-1 input+bias matmuls for step t+1 are issued during step t into the
    other PSUM buffer (bufs=2), so only the 4 hidden matmuls precede the gate
    activation on the critical path.
"""

import os
import numpy as np

import concourse.bass as bass
import concourse.mybir as mybir
from concourse import bacc
from concourse.tile import TileContext
from concourse.bass_utils import run_bass_kernel_spmd

N_CORES = 8
B, T, D = 2048, 128, 128
H1, H2, OUT = 128, 64, 64
BC = B // N_CORES  # 256 batch per core
X_CHUNKS = [(0, 8), (8, 24), (24, 56), (56, 128)]  # staged x DMA (ramp-friendly)

FP = mybir.dt.float32
F16 = mybir.dt.float16
AF = mybir.ActivationFunctionType
ALU = mybir.AluOpType

_PROGRAM_CACHE = {}


def build_program():
    if "nc" in _PROGRAM_CACHE:
        return _PROGRAM_CACHE["nc"]

    nc = bacc.Bacc(
        "TRN2", target_bir_lowering=False, debug=False,
        enable_asserts=False, num_devices=N_CORES,
    )

    # ---- DRAM parameters (per-core shapes; in_maps supply per-core data)
    xT_d = nc.declare_dram_parameter("xT", [D, T, BC], F16, isOutput=False)
    w1_d = nc.declare_dram_parameter("w1", [D, 4 * H1], F16, isOutput=False)
    wh1_d = nc.declare_dram_parameter("wh1", [H1, 4 * H1], F16, isOutput=False)
    b1_d = nc.declare_dram_parameter("b1", [1, 4 * H1], F16, isOutput=False)
    w2_d = nc.declare_dram_parameter("w2", [H1, 2 * 2 * H2], F16, isOutput=False)
    wh2_d = nc.declare_dram_parameter("wh2", [H2, 2 * 2 * H2], F16, isOutput=False)
    b2_d = nc.declare_dram_parameter("b2", [1, 2 * 2 * H2], F16, isOutput=False)
    wd_d = nc.declare_dram_parameter("wd", [H2, OUT], F16, isOutput=False)
    bd_d = nc.declare_dram_parameter("bd", [1, OUT], F16, isOutput=False)
    ones_d = nc.declare_dram_parameter("ones", [1, BC], F16, isOutput=False)
    out_d = nc.declare_dram_parameter("outT", [OUT, BC], FP, isOutput=True)

    with TileContext(nc, num_cores=N_CORES) as tc:
        with (
            tc.tile_pool(name="const", bufs=1) as cpool,
            tc.tile_pool(name="acts", bufs=4) as apool,
            tc.tile_pool(name="state", bufs=4) as spool,
            tc.tile_pool(name="ps1", bufs=2, space="PSUM") as ps1pool,
            tc.tile_pool(name="ps2", bufs=2, space="PSUM") as ps2pool,
            tc.tile_pool(name="psd", bufs=1, space="PSUM") as psdpool,
        ):
            # ---- load constants / weights / x into SBUF
            w1 = cpool.tile([D, 4 * H1], F16, tag="w1")
            wh1 = cpool.tile([H1, 4 * H1], F16, tag="wh1")
            b1 = cpool.tile([1, 4 * H1], F16, tag="b1")
            w2 = cpool.tile([H1, 2, 2 * H2], F16, tag="w2")
            wh2 = cpool.tile([H2, 2, 2 * H2], F16, tag="wh2")
            b2 = cpool.tile([1, 2, 2 * H2], F16, tag="b2")
            wd = cpool.tile([H2, OUT], F16, tag="wd")
            bd = cpool.tile([1, OUT], F16, tag="bd")
            ones = cpool.tile([1, BC], F16, tag="ones")
            for sb, dr in ((w1, w1_d), (wh1, wh1_d), (b1, b1_d), (w2, w2_d),
                           (wh2, wh2_d), (b2, b2_d), (wd, wd_d), (bd, bd_d),
                           (ones, ones_d)):
                nc.sync.dma_start(out=sb[:], in_=dr[:])

            xs = cpool.tile([D, T, BC], F16, tag="xs")
            for a, b_ in X_CHUNKS:
                nc.sync.dma_start(out=xs[:, a:b_, :], in_=xT_d[:, a:b_, :])

            czero = cpool.tile([H1, BC], F16, tag="czero")
            nc.vector.memset(czero[:], 0.0)

            h1p = c1p = h2p = c2p = None  # previous-step states

            def l1_prefetch(t):
                """input+bias matmuls for L1 step t into a fresh PSUM buffer."""
                p = ps1pool.tile([H1, 4, BC], FP, tag="p1")
                xt = xs[:, t, :]
                last = t == 0  # no hidden matmuls at t=0 -> stops live here
                for j in range(4):
                    nc.tensor.matmul(p[:, j, :], w1[:, j * H1:(j + 1) * H1],
                                     xt, start=(j in (0, 2)), stop=False)
                for j in range(4):
                    nc.tensor.matmul(p[:, j, :], b1[:, j * H1:(j + 1) * H1],
                                     ones[:], start=False,
                                     stop=(last and j in (1, 3)))
                return p

            def l2_matmuls(s, h1s):
                """all matmuls for L2 step s (input from h1s, hidden from h2p)."""
                p = ps2pool.tile([2 * H2, 2, BC], FP, tag="p2")
                for k in range(2):
                    nc.tensor.matmul(p[:, k, :], w2[:, k, :], h1s[:],
                                     start=(k == 0), stop=False)
                for k in range(2):
                    nc.tensor.matmul(p[:, k, :], b2[:, k, :], ones[:],
                                     start=False, stop=(s == 0 and k == 1))
                if s > 0:
                    for k in range(2):
                        nc.tensor.matmul(p[:, k, :], wh2[:, k, :], h2p[:],
                                         start=False, stop=(k == 1))
                return p

            p1 = l1_prefetch(0)
            for t in range(T + 1):
                s = t - 1  # L2 step handled this iteration
                if t < T:
                    # -- PE: L1 hidden matmuls for step t (chain-critical).
                    # Gate order (f,g | i,o): bank0 closes after 2 matmuls so
                    # the first gate-ACT half starts early.
                    if t > 0:
                        for j in range(4):
                            nc.tensor.matmul(p1[:, j, :], wh1[:, j * H1:(j + 1) * H1],
                                             h1p[:], start=False, stop=(j in (1, 3)))

                    # -- ACT: L1 gates, split in two halves [f,g] then [i,o]
                    sg = apool.tile([H1, 4, BC], F16, tag="sg")
                    nc.scalar.activation(sg[:, 0:2, :], p1[:, 0:2, :], AF.Sigmoid)
                    nc.scalar.activation(sg[:, 2:4, :], p1[:, 2:4, :], AF.Sigmoid)

                # -- PE: all matmuls for L2 step s (off-chain)
                if s >= 0:
                    p2 = l2_matmuls(s, h1p)

                if t < T:
                    # -- DVE: L1 cell update (gt = tanh(g) = 2*sg_g - 1)
                    gt = apool.tile([H1, BC], F16, tag="gt")
                    v = apool.tile([H1, BC], F16, tag="v")
                    u = apool.tile([H1, BC], F16, tag="u")
                    c1n = spool.tile([H1, BC], F16, tag="c1")
                    nc.vector.tensor_scalar(gt[:], sg[:, 1, :], 2.0, 1.0,
                                            op0=ALU.mult, op1=ALU.subtract)
                    nc.vector.tensor_tensor(v[:], sg[:, 0, :],
                                            czero[:] if t == 0 else c1p[:], op=ALU.mult)
                    nc.vector.tensor_tensor(u[:], sg[:, 2, :], gt[:], op=ALU.mult)
                    nc.vector.tensor_tensor(c1n[:], u[:], v[:], op=ALU.add)

                if s >= 0:
                    # -- ACT: L2 gates
                    sg2 = apool.tile([H1, 2, BC], F16, tag="sg2")
                    nc.scalar.activation(sg2[:], p2[:], AF.Sigmoid)
                    # -- POOL: v2 (off the DVE queue)
                    v2 = apool.tile([H2, BC], F16, tag="v2")
                    nc.gpsimd.tensor_tensor(v2[:], sg2[0:H2, 1, :],
                                            czero[0:H2, :] if s == 0 else c2p[:],
                                            op=ALU.mult)
                    # -- DVE: gt2, u2
                    gt2 = apool.tile([H2, BC], F16, tag="gt2")
                    u2 = apool.tile([H2, BC], F16, tag="u2")
                    nc.vector.tensor_scalar(gt2[:], sg2[H2:2 * H2, 0, :], 2.0, 1.0,
                                            op0=ALU.mult, op1=ALU.subtract)
                    nc.vector.tensor_tensor(u2[:], sg2[0:H2, 0, :], gt2[:], op=ALU.mult)

                if t < T:
                    # -- ACT: thc1 (chain), then DVE: h1n (chain)
                    thc1 = apool.tile([H1, BC], F16, tag="thc1")
                    nc.scalar.activation(thc1[:], c1n[:], AF.Tanh)
                    h1n = spool.tile([H1, BC], F16, tag="h1")
                    nc.vector.tensor_tensor(h1n[:], sg[:, 3, :], thc1[:], op=ALU.mult)

                if s >= 0:
                    # -- DVE: c2n; ACT: thc2; DVE: h2n
                    c2n = spool.tile([H2, BC], F16, tag="c2")
                    nc.vector.tensor_tensor(c2n[:], u2[:], v2[:], op=ALU.add)
                    thc2 = apool.tile([H2, BC], F16, tag="thc2")
                    nc.scalar.activation(thc2[:], c2n[:], AF.Tanh)
                    h2n = spool.tile([H2, BC], F16, tag="h2")
                    nc.vector.tensor_tensor(h2n[:], sg2[H2:2 * H2, 1, :], thc2[:],
                                            op=ALU.mult)
                    h2p, c2p = h2n, c2n

                # -- PE: prefetch L1 input+bias for step t+1
                if t < T - 1:
                    p1_next = l1_prefetch(t + 1)

                if t < T:
                    c1p, h1p = c1n, h1n
                if t < T - 1:
                    p1 = p1_next

            # ---- dense head on h2[T-1]
            pd = psdpool.tile([OUT, BC], FP, tag="pd")
            nc.tensor.matmul(pd[:], wd[:], h2p[:], start=True, stop=False)
            nc.tensor.matmul(pd[:], bd[:], ones[:], start=False, stop=True)
            outs = cpool.tile([OUT, BC], FP, tag="outs")
            nc.scalar.activation(outs[:], pd[:], AF.Relu)
            nc.sync.dma_start(out=out_d[:], in_=outs[:])

    nc.finalize()
    _PROGRAM_CACHE["nc"] = nc
    return nc


def _prep_inputs(x, w_ih1, w_hh1, b_ih1, b_hh1, w_ih2, w_hh2, b_ih2, b_hh2,
                 w_dense, b_dense):
    """Host-side layout prep (fp16). Device gate order: [i, f, o, 2g] for L1;
    packed [i | 2g], [f | o] columns for L2. g weights doubled because
    tanh(z) = 2*sigmoid(2z) - 1 on device."""
    f16 = np.float16

    def gates(w_t, H):  # w_t: [in, 4H] torch order (i,f,g,o)
        i, f, g, o = (np.float64(w_t[:, k * H:(k + 1) * H]) for k in range(4))
        return i, f, 2.0 * g, o

    def cat(parts):
        return np.concatenate(parts, axis=-1).astype(f16)

    i1, f1, g1, o1 = gates(w_ih1.T, H1)
    w1 = cat([f1, g1, i1, o1])
    i1, f1, g1, o1 = gates(w_hh1.T, H1)
    wh1 = cat([f1, g1, i1, o1])
    i1, f1, g1, o1 = gates((b_ih1 + b_hh1)[None, :], H1)
    b1 = cat([f1, g1, i1, o1])

    i2, f2, g2, o2 = gates(w_ih2.T, H2)
    w2 = cat([i2, g2, f2, o2])
    i2, f2, g2, o2 = gates(w_hh2.T, H2)
    wh2 = cat([i2, g2, f2, o2])
    i2, f2, g2, o2 = gates((b_ih2 + b_hh2)[None, :], H2)
    b2 = cat([i2, g2, f2, o2])

    wd = np.float64(w_dense.T).astype(f16)
    bd = b_dense.astype(f16)[None, :]

    xT = np.asarray(x, dtype=f16).transpose(2, 1, 0)  # [D,T,B]
    shared = dict(w1=w1, wh1=wh1, b1=b1, w2=w2, wh2=wh2, b2=b2, wd=wd, bd=bd,
                  ones=np.ones((1, BC), f16))
    in_maps = []
    for c in range(N_CORES):
        m = dict(shared)
        m["xT"] = np.ascontiguousarray(xT[:, :, c * BC:(c + 1) * BC])
        in_maps.append(m)
    return in_maps


def _run(inputs, trace=False, **kw):
    nc = build_program()
    in_maps = _prep_inputs(**inputs)
    res = run_bass_kernel_spmd(nc, in_maps, list(range(N_CORES)), trace=trace, **kw)
    out = np.concatenate([np.asarray(res.results[c]["outT"]).T for c in range(N_CORES)], axis=0)
    return out.astype(np.float32), res


def kernel(**inputs):
    out, _ = _run(inputs, trace=False)
    return out


if __name__ == "__main__":
    import reference
    inputs = {k: np.asarray(v) for k, v in reference.setup_inputs().items()}
    expected = np.asarray(reference.reference(**inputs))
    out, res = _run(inputs, trace=os.environ.get("KTRACE", "0") == "1")
    err = np.abs(out - expected)
    rel = err.max() / (np.abs(expected).max() + 1e-12)
    print("max abs err:", err.max(), "rel:", rel)
    print("exec_time_ns:", res.exec_time_ns)


# revision 17
# speedup vs baseline: 1.1510x; 1.1510x over previous
"""Trainium2 Bass kernel for a 2-layer LSTM + dense head (batch-sharded over 8 cores).

Reference computation (PyTorch gate order i,f,g,o):
  h1 = LSTM(x;   w_ih1, w_hh1, b_ih1+b_hh1)   # D=128 -> H1=128
  h2 = LSTM(h1;  w_ih2, w_hh2, b_ih2+b_hh2)   # H1=128 -> H2=64
  out = relu(h2[:, -1] @ w_dense.T + b_dense) # [B, 64]

Device-side design (per core, B_c = 256 batch columns):
  - All state "transposed": hidden-dim on SBUF partitions, batch on free dim.
  - fp16 everywhere in SBUF (x, weights, states, gate outputs); fp32 in PSUM.
    fp16 matmuls run at 1 cycle/row; fp16 doubles DVE tensor_tensor rate.
  - Gates i,f,o use Sigmoid directly; the g gate's tanh is computed as
    tanh(z) = 2*sigmoid(2z) - 1 (g weights doubled on host), so ALL four
    gates go through a single Sigmoid ACT op per layer, and every
    elementwise op is a 2-input tensor_tensor (fp16 2x on DVE):
      sg    = sigmoid(psum[i | f | o | 2g])   # one ACT op [128,1024]
      gt    = 2*sg_g - 1                      # tensor_scalar = tanh(g)
      v     = sg_f * c ;  u = sg_i * gt ;  c' = u + v
      thc   = tanh(c') ;  h' = sg_o * thc
    (Sigmoid, Tanh, Relu all live in one HW activation table -> no reloads.)
  - Layer 2 runs one timestep BEHIND layer 1 (software pipeline) so the two
    recurrence chains overlap.  Layer 2's four 64-row gates are packed
    pairwise into two 128-row matmuls: colA = [i | 2g], colB = [f | o].
    Its v2 multiply runs on GPSIMD to keep the DVE/ACT queues tight.
  - Layer-1 input+bias matmuls for step t+1 are issued during step t into the
    other PSUM buffer (bufs=2), so only the 4 hidden matmuls precede the gate
    activation on the critical path.
"""

import os
import numpy as np

import concourse.bass as bass
import concourse.mybir as mybir
from concourse import bacc
from concourse.tile import TileContext
from concourse.bass_utils import run_bass_kernel_spmd

N_CORES = 8
B, T, D = 2048, 128, 128
H1, H2, OUT = 128, 64, 64
BC = B // N_CORES  # 256 batch per core
X_CHUNKS = [(0, 8), (8, 24), (24, 56), (56, 128)]  # staged x DMA (ramp-friendly)

FP = mybir.dt.float32
F16 = mybir.dt.float16
AF = mybir.ActivationFunctionType
ALU = mybir.AluOpType

_PROGRAM_CACHE = {}


def build_program():
    if "nc" in _PROGRAM_CACHE:
        return _PROGRAM_CACHE["nc"]

    nc = bacc.Bacc(
        "TRN2", target_bir_lowering=False, debug=False,
        enable_asserts=False, num_devices=N_CORES,
    )

    # ---- DRAM parameters (per-core shapes; in_maps supply per-core data)
    xT_d = nc.declare_dram_parameter("xT", [D, T, BC], F16, isOutput=False)
    w1_d = nc.declare_dram_parameter("w1", [D, 4 * H1], F16, isOutput=False)
    wh1_d = nc.declare_dram_parameter("wh1", [H1, 4 * H1], F16, isOutput=False)
    b1_d = nc.declare_dram_parameter("b1", [1, 4 * H1], F16, isOutput=False)
    w2_d = nc.declare_dram_parameter("w2", [H1, 2 * 2 * H2], F16, isOutput=False)
    wh2_d = nc.declare_dram_parameter("wh2", [H2, 2 * 2 * H2], F16, isOutput=False)
    b2_d = nc.declare_dram_parameter("b2", [1, 2 * 2 * H2], F16, isOutput=False)
    wd_d = nc.declare_dram_parameter("wd", [H2, OUT], F16, isOutput=False)
    bd_d = nc.declare_dram_parameter("bd", [1, OUT], F16, isOutput=False)
    ones_d = nc.declare_dram_parameter("ones", [1, BC], F16, isOutput=False)
    out_d = nc.declare_dram_parameter("outT", [OUT, BC], FP, isOutput=True)

    with TileContext(nc, num_cores=N_CORES) as tc:
        with (
            tc.tile_pool(name="const", bufs=1) as cpool,
            tc.tile_pool(name="acts", bufs=4) as apool,
            tc.tile_pool(name="state", bufs=4) as spool,
            tc.tile_pool(name="ps1", bufs=2, space="PSUM") as ps1pool,
            tc.tile_pool(name="ps2", bufs=2, space="PSUM") as ps2pool,
            tc.tile_pool(name="psd", bufs=1, space="PSUM") as psdpool,
        ):
            # ---- load constants / weights / x into SBUF
            w1 = cpool.tile([D, 4 * H1], F16, tag="w1")
            wh1 = cpool.tile([H1, 4 * H1], F16, tag="wh1")
            b1 = cpool.tile([1, 4 * H1], F16, tag="b1")
            w2 = cpool.tile([H1, 2, 2 * H2], F16, tag="w2")
            wh2 = cpool.tile([H2, 2, 2 * H2], F16, tag="wh2")
            b2 = cpool.tile([1, 2, 2 * H2], F16, tag="b2")
            wd = cpool.tile([H2, OUT], F16, tag="wd")
            bd = cpool.tile([1, OUT], F16, tag="bd")
            ones = cpool.tile([1, BC], F16, tag="ones")
            for sb, dr in ((w1, w1_d), (wh1, wh1_d), (b1, b1_d), (w2, w2_d),
                           (wh2, wh2_d), (b2, b2_d), (wd, wd_d), (bd, bd_d),
                           (ones, ones_d)):
                nc.sync.dma_start(out=sb[:], in_=dr[:])

            xs = cpool.tile([D, T, BC], F16, tag="xs")
            for a, b_ in X_CHUNKS:
                nc.sync.dma_start(out=xs[:, a:b_, :], in_=xT_d[:, a:b_, :])

            czero = cpool.tile([H1, BC], F16, tag="czero")
            nc.vector.memset(czero[:], 0.0)

            h1p = c1p = h2p = c2p = None  # previous-step states

            def l1_prefetch(t):
                """input+bias matmuls for L1 step t into a fresh PSUM buffer."""
                p = ps1pool.tile([H1, 4, BC], FP, tag="p1")
                xt = xs[:, t, :]
                last = t == 0  # no hidden matmuls at t=0 -> stops live here
                for j in range(4):
                    nc.tensor.matmul(p[:, j, :], w1[:, j * H1:(j + 1) * H1],
                                     xt, start=(j in (0, 2)), stop=False)
                for j in range(4):
                    nc.tensor.matmul(p[:, j, :], b1[:, j * H1:(j + 1) * H1],
                                     ones[:], start=False,
                                     stop=(last and j in (1, 3)))
                return p

            def l2_matmuls(s, h1s):
                """all matmuls for L2 step s (input from h1s, hidden from h2p)."""
                p = ps2pool.tile([2 * H2, 2, BC], FP, tag="p2")
                for k in range(2):
                    nc.tensor.matmul(p[:, k, :], w2[:, k, :], h1s[:],
                                     start=(k == 0), stop=False)
                for k in range(2):
                    nc.tensor.matmul(p[:, k, :], b2[:, k, :], ones[:],
                                     start=False, stop=(s == 0 and k == 1))
                if s > 0:
                    for k in range(2):
                        nc.tensor.matmul(p[:, k, :], wh2[:, k, :], h2p[:],
                                         start=False, stop=(k == 1))
                return p

            p1 = l1_prefetch(0)
            for t in range(T + 1):
                s = t - 1  # L2 step handled this iteration
                if t < T:
                    # -- PE: L1 hidden matmuls for step t (chain-critical).
                    # Gate order (f, g, i, o).
                    if t > 0:
                        for j in range(4):
                            nc.tensor.matmul(p1[:, j, :], wh1[:, j * H1:(j + 1) * H1],
                                             h1p[:], start=False, stop=(j in (1, 3)))

                    # -- ACT: L1 gates (one sigmoid over all four)
                    sg = apool.tile([H1, 4, BC], F16, tag="sg")
                    nc.scalar.activation(sg[:], p1[:], AF.Sigmoid)

                # -- PE: all matmuls for L2 step s (off-chain)
                if s >= 0:
                    p2 = l2_matmuls(s, h1p)

                if t < T:
                    # -- DVE: L1 cell update (gt = tanh(g) = 2*sg_g - 1)
                    gt = apool.tile([H1, BC], F16, tag="gt")
                    v = apool.tile([H1, BC], F16, tag="v")
                    u = apool.tile([H1, BC], F16, tag="u")
                    c1n = spool.tile([H1, BC], F16, tag="c1")
                    nc.vector.tensor_scalar(gt[:], sg[:, 1, :], 2.0, 1.0,
                                            op0=ALU.mult, op1=ALU.subtract)
                    nc.vector.tensor_tensor(v[:], sg[:, 0, :],
                                            czero[:] if t == 0 else c1p[:], op=ALU.mult)
                    nc.vector.tensor_tensor(u[:], sg[:, 2, :], gt[:], op=ALU.mult)
                    nc.vector.tensor_tensor(c1n[:], u[:], v[:], op=ALU.add)

                if s >= 0:
                    # -- ACT: L2 gates
                    sg2 = apool.tile([H1, 2, BC], F16, tag="sg2")
                    nc.scalar.activation(sg2[:], p2[:], AF.Sigmoid)
                    # -- POOL: v2 (off the DVE queue)
                    v2 = apool.tile([H2, BC], F16, tag="v2")
                    nc.gpsimd.tensor_tensor(v2[:], sg2[0:H2, 1, :],
                                            czero[0:H2, :] if s == 0 else c2p[:],
                                            op=ALU.mult)
                    # -- DVE: gt2, u2
                    gt2 = apool.tile([H2, BC], F16, tag="gt2")
                    u2 = apool.tile([H2, BC], F16, tag="u2")
                    nc.vector.tensor_scalar(gt2[:], sg2[H2:2 * H2, 0, :], 2.0, 1.0,
                                            op0=ALU.mult, op1=ALU.subtract)
                    nc.vector.tensor_tensor(u2[:], sg2[0:H2, 0, :], gt2[:], op=ALU.mult)

                if t < T:
                    # -- ACT: thc1 (chain), then DVE: h1n (chain)
                    thc1 = apool.tile([H1, BC], F16, tag="thc1")
                    nc.scalar.activation(thc1[:], c1n[:], AF.Tanh)
                    h1n = spool.tile([H1, BC], F16, tag="h1")
                    nc.vector.tensor_tensor(h1n[:], sg[:, 3, :], thc1[:], op=ALU.mult)

                if s >= 0:
                    # -- DVE: c2n; ACT: thc2; DVE: h2n
                    c2n = spool.tile([H2, BC], F16, tag="c2")
                    nc.vector.tensor_tensor(c2n[:], u2[:], v2[:], op=ALU.add)
                    thc2 = apool.tile([H2, BC], F16, tag="thc2")
                    nc.scalar.activation(thc2[:], c2n[:], AF.Tanh)
                    h2n = spool.tile([H2, BC], F16, tag="h2")
                    nc.vector.tensor_tensor(h2n[:], sg2[H2:2 * H2, 1, :], thc2[:],
                                            op=ALU.mult)
                    h2p, c2p = h2n, c2n

                # -- PE: prefetch L1 input+bias for step t+1
                if t < T - 1:
                    p1_next = l1_prefetch(t + 1)

                if t < T:
                    c1p, h1p = c1n, h1n
                if t < T - 1:
                    p1 = p1_next

            # ---- dense head on h2[T-1]
            pd = psdpool.tile([OUT, BC], FP, tag="pd")
            nc.tensor.matmul(pd[:], wd[:], h2p[:], start=True, stop=False)
            nc.tensor.matmul(pd[:], bd[:], ones[:], start=False, stop=True)
            outs = cpool.tile([OUT, BC], FP, tag="outs")
            nc.scalar.activation(outs[:], pd[:], AF.Relu)
            nc.sync.dma_start(out=out_d[:], in_=outs[:])

    nc.finalize()
    _PROGRAM_CACHE["nc"] = nc
    return nc


def _prep_inputs(x, w_ih1, w_hh1, b_ih1, b_hh1, w_ih2, w_hh2, b_ih2, b_hh2,
                 w_dense, b_dense):
    """Host-side layout prep (fp16). Device gate order: [i, f, o, 2g] for L1;
    packed [i | 2g], [f | o] columns for L2. g weights doubled because
    tanh(z) = 2*sigmoid(2z) - 1 on device."""
    f16 = np.float16

    def gates(w_t, H):  # w_t: [in, 4H] torch order (i,f,g,o)
        i, f, g, o = (np.float64(w_t[:, k * H:(k + 1) * H]) for k in range(4))
        return i, f, 2.0 * g, o

    def cat(parts):
        return np.concatenate(parts, axis=-1).astype(f16)

    i1, f1, g1, o1 = gates(w_ih1.T, H1)
    w1 = cat([f1, g1, i1, o1])
    i1, f1, g1, o1 = gates(w_hh1.T, H1)
    wh1 = cat([f1, g1, i1, o1])
    i1, f1, g1, o1 = gates((b_ih1 + b_hh1)[None, :], H1)
    b1 = cat([f1, g1, i1, o1])

    i2, f2, g2, o2 = gates(w_ih2.T, H2)
    w2 = cat([i2, g2, f2, o2])
    i2, f2, g2, o2 = gates(w_hh2.T, H2)
    wh2 = cat([i2, g2, f2, o2])
    i2, f2, g2, o2 = gates((b_ih2 + b_hh2)[None, :], H2)
    b2 = cat([i2, g2, f2, o2])

    wd = np.float64(w_dense.T).astype(f16)
    bd = b_dense.astype(f16)[None, :]

    xT = np.asarray(x, dtype=f16).transpose(2, 1, 0)  # [D,T,B]
    shared = dict(w1=w1, wh1=wh1, b1=b1, w2=w2, wh2=wh2, b2=b2, wd=wd, bd=bd,
                  ones=np.ones((1, BC), f16))
    in_maps = []
    for c in range(N_CORES):
        m = dict(shared)
        m["xT"] = np.ascontiguousarray(xT[:, :, c * BC:(c + 1) * BC])
        in_maps.append(m)
    return in_maps


def _run(inputs, trace=False, **kw):
    nc = build_program()
    in_maps = _prep_inputs(**inputs)
    res = run_bass_kernel_spmd(nc, in_maps, list(range(N_CORES)), trace=trace, **kw)
    out = np.concatenate([np.asarray(res.results[c]["outT"]).T for c in range(N_CORES)], axis=0)
    return out.astype(np.float32), res


def kernel(**inputs):
    out, _ = _run(inputs, trace=False)
    return out


if __name__ == "__main__":
    import reference
    inputs = {k: np.asarray(v) for k, v in reference.setup_inputs().items()}
    expected = np.asarray(reference.reference(**inputs))
    out, res = _run(inputs, trace=os.environ.get("KTRACE", "0") == "1")
    err = np.abs(out - expected)
    rel = err.max() / (np.abs(expected).max() + 1e-12)
    print("max abs err:", err.max(), "rel:", rel)
    print("exec_time_ns:", res.exec_time_ns)
